# revision 17
# baseline (speedup 1.0000x reference)
"""GATv2 mini-model (2-layer bipartite GATv2 message passing) on 8 Trainium2 NeuronCores.

Sharding: edges are partitioned by destination-node blocks across the 8 cores
(1-D graph partition).  Each core owns a contiguous block of dst nodes for both
node types, processes the incoming edges of those nodes (sorted by dst,
grouped into 127-dst chunks and 128-edge tiles), and produces the new features
of its dst block.  Layer-1 source features are host-pre-gathered from the raw
inputs (indices are known on the host); layer-2 source features are fetched
from AllGather-replicated node tables with indirect-DMA row gathers.  Dense
per-node linears are data-parallel over nodes.

Per edge tile the attention math is expressed as matmuls against a one-hot
dst-selection matrix; the 128th (unused) one-hot slot carries the per-edge
scalar edge weight so the edge-embedding rank-1 term rides along the xr
matmul for free.
"""

import sys

sys.path.insert(0, "/opt/trn_rl_repo")

from dataclasses import dataclass

import numpy as np

from concourse import bacc, bass, mybir
import concourse.tile as tile

P = 128
CW = 127  # dst chunk width (slot 127 of the one-hot carries the edge scalar)
F32 = mybir.dt.float32
F32R = mybir.dt.float32r
I32 = mybir.dt.int32
AF = mybir.ActivationFunctionType
OP = mybir.AluOpType
NEG_SLOPE = 0.2


@dataclass
class Cfg:
    Nu: int = 100000
    Ni: int = 50000
    Du: int = 128
    Di: int = 64
    Hd: int = 128
    H: int = 4
    C: int = 128
    M: int = 8  # cores

    @property
    def HC(self):
        return self.H * self.C

    @property
    def blk_u(self):
        return self.Nu // self.M

    @property
    def blk_i(self):
        return self.Ni // self.M


@dataclass
class DirSched:
    tiles: list   # per-chunk edge-tile count (uniform across cores)
    widths: list  # per-chunk dst width (<= 127)
    T: int        # total edge tiles per core

    @property
    def starts(self):
        s = [0]
        for t in self.tiles:
            s.append(s[-1] + t)
        return s


def _prep_direction(src, dst, ea, n_dst, M):
    """Sort edges by dst, partition into per-core dst blocks and 127-dst
    chunks, pad each (core, chunk) to a core-uniform tile count."""
    blk = n_dst // M
    n_chunks = -(-blk // CW)
    order = np.argsort(dst, kind="stable")
    src_s = np.asarray(src)[order].astype(np.int64)
    dst_s = np.asarray(dst)[order].astype(np.int64)
    ea_s = np.asarray(ea).reshape(-1)[order].astype(np.float32)

    core = dst_s // blk
    loc = dst_s % blk
    chunk = loc // CW
    dstl = (loc % CW).astype(np.float32)
    # within a (core, chunk) run edges are already contiguous because the sort
    # is by global dst and (core, chunk, dstl) is monotone in dst
    gcid = core * n_chunks + chunk
    counts = np.bincount(gcid, minlength=M * n_chunks).reshape(M, n_chunks)
    tiles = np.maximum(1, -(-counts // P)).max(axis=0)
    T = int(tiles.sum())
    starts = np.concatenate([[0], np.cumsum(tiles)]).astype(np.int64)
    run_starts = np.concatenate([[0], np.cumsum(counts.reshape(-1))]).astype(np.int64)

    gidx = np.zeros((M, T * P), np.int64)
    dstc = np.full((M, T * P), -1.0, np.float32)
    er = np.zeros((M, T * P), np.float32)
    for c in range(M):
        for j in range(n_chunks):
            k = c * n_chunks + j
            e0, e1 = run_starts[k], run_starts[k + 1]
            n = e1 - e0
            o = int(starts[j]) * P
            gidx[c, o : o + n] = src_s[e0:e1]
            dstc[c, o : o + n] = dstl[e0:e1]
            er[c, o : o + n] = ea_s[e0:e1]

    widths = [min(CW, blk - j * CW) for j in range(n_chunks)]
    sched = DirSched(tiles=[int(t) for t in tiles], widths=widths, T=T)
    return gidx, dstc, er, sched


def prep_inputs(inputs, cfg: Cfg):
    """Host-side preprocessing: edge sorting/padding, layer-1 pre-gather,
    per-core in_maps."""
    M = cfg.M
    f = lambda k: np.asarray(inputs[k], dtype=np.float32)

    gidx_R, dstc_R, er_R, sr = _prep_direction(
        np.asarray(inputs["src_rates"]), np.asarray(inputs["dst_rates"]),
        np.asarray(inputs["ea_rates"]), cfg.Ni, M)
    gidx_D, dstc_D, er_D, sd = _prep_direction(
        np.asarray(inputs["src_rated"]), np.asarray(inputs["dst_rated"]),
        np.asarray(inputs["ea_rated"]), cfg.Nu, M)

    x_user = f("x_user")
    x_item = f("x_item")
    # scale er by exp(logw) once here (per relation)
    er_R = er_R * np.exp(np.float32(np.asarray(inputs["logw_rates"])))
    er_D = er_D * np.exp(np.float32(np.asarray(inputs["logw_rated"])))

    n_ch_R = len(sr.tiles)   # item-dst chunks (xr table width for rates)
    n_ch_D = len(sd.tiles)   # user-dst chunks
    Wi_tab = (n_ch_R * CW + 2) // 2 * 2
    Wu_tab = (n_ch_D * CW + 2) // 2 * 2

    shared = {}
    shared["bpc_user"] = f("bp_user").reshape(-1, 1)
    shared["bpc_item"] = f("bp_item").reshape(-1, 1)
    shared["Wp_user"] = f("Wp_user")
    shared["Wp_item"] = f("Wp_item")
    for r in ("rates", "rated"):
        for l in range(2):
            shared[f"Wl_{r}{l}"] = f(f"Wl_{r}")[l]
            shared[f"Wr_{r}{l}"] = f(f"Wr_{r}")[l]
            shared[f"blbr_{r}{l}"] = (f(f"bl_{r}")[l] + f(f"br_{r}")[l]).reshape(1, -1)
            shared[f"bl_{r}{l}"] = f(f"bl_{r}")[l].reshape(1, -1)
            shared[f"We_{r}{l}"] = f(f"We_{r}")[l].reshape(1, -1)
            shared[f"att_{r}{l}"] = f(f"att_{r}")[l].reshape(1, -1)
            shared[f"bias_{r}{l}"] = f(f"bias_{r}")[l].reshape(1, -1)
    shared["Wf_user"] = f("Wf_user")
    shared["bf_user"] = f("bf_user").reshape(1, -1)
    shared["Wf_item"] = f("Wf_item")
    shared["bf_item"] = f("bf_item").reshape(1, -1)

    in_maps = []
    for c in range(M):
        m = dict(shared)
        xu = x_user[c * cfg.blk_u : (c + 1) * cfg.blk_u]
        xi = x_item[c * cfg.blk_i : (c + 1) * cfg.blk_i]
        xT_u = np.zeros((cfg.Du, Wu_tab), np.float32)
        xT_u[:, : cfg.blk_u] = xu.T
        xT_i = np.zeros((cfg.Di, Wi_tab), np.float32)
        xT_i[:, : cfg.blk_i] = xi.T
        m["xT_user_sh"] = xT_u
        m["xT_item_sh"] = xT_i
        for r, gidx, dstc, er, s, xsrc, Dx in (
            ("rates", gidx_R, dstc_R, er_R, sr, x_user, cfg.Du),
            ("rated", gidx_D, dstc_D, er_D, sd, x_item, cfg.Di),
        ):
            T = s.T
            m[f"gidx_{r}"] = np.ascontiguousarray(
                gidx[c].astype(np.int32).reshape(T, P).T)
            m[f"dstc_{r}"] = np.ascontiguousarray(dstc[c].reshape(T, P).T)
            m[f"erc_{r}"] = np.ascontiguousarray(er[c].reshape(T, P).T)
            m[f"xgT_{r}"] = np.ascontiguousarray(xsrc[gidx[c]].T)  # [Dx, T*P]
        in_maps.append(m)
    return in_maps, sr, sd


def build_program(cfg: Cfg, sr: DirSched, sd: DirSched):
    nc = bacc.Bacc("TRN2", target_bir_lowering=False, debug=False,
                   enable_asserts=False, num_devices=cfg.M)
    Hd, H, C, HC = cfg.Hd, cfg.H, cfg.C, cfg.HC
    blk_u, blk_i = cfg.blk_u, cfg.blk_i
    n_ch_R, n_ch_D = len(sr.tiles), len(sd.tiles)
    Wi_tab, Wu_tab = (n_ch_R * CW + 2) // 2 * 2, (n_ch_D * CW + 2) // 2 * 2

    def r32(ap):
        return ap.bitcast(F32R)

    # ---- I/O ----
    ein = lambda n, s, dt=F32: nc.dram_tensor(n, s, dt, kind="ExternalInput").ap()
    xT_user = ein("xT_user_sh", [cfg.Du, Wu_tab], F32R)
    xT_item = ein("xT_item_sh", [cfg.Di, Wi_tab], F32R)
    gmeta = {}
    for r, s, Dx in (("rates", sr, cfg.Du), ("rated", sd, cfg.Di)):
        gmeta[r] = dict(
            gidx=ein(f"gidx_{r}", [P, s.T], I32),
            dstc=ein(f"dstc_{r}", [P, s.T]),
            erc=ein(f"erc_{r}", [P, s.T]),
            xgT=ein(f"xgT_{r}", [Dx, s.T * P], F32R),
        )
    wd_in = {}
    for name, shape, dt_ in [
        ("Wp_user", [cfg.Du, Hd], F32R), ("bpc_user", [Hd, 1], F32),
        ("Wp_item", [cfg.Di, Hd], F32R), ("bpc_item", [Hd, 1], F32),
        ("Wf_user", [Hd, Hd], F32R), ("bf_user", [1, Hd], F32),
        ("Wf_item", [Hd, Hd], F32R), ("bf_item", [1, Hd], F32),
    ]:
        wd_in[name] = ein(name, shape, dt_)
    for r in ("rates", "rated"):
        for l in range(2):
            for name, shape, dt_ in [
                (f"Wl_{r}{l}", [Hd, HC], F32R), (f"Wr_{r}{l}", [Hd, HC], F32R),
                (f"blbr_{r}{l}", [1, HC], F32), (f"We_{r}{l}", [1, HC], F32),
                (f"bl_{r}{l}", [1, HC], F32),
                (f"att_{r}{l}", [1, HC], F32), (f"bias_{r}{l}", [1, C], F32),
            ]:
                wd_in[name] = ein(name, shape, dt_)
    zu = nc.dram_tensor("zu_sh", [blk_u, Hd], F32, kind="ExternalOutput").ap()
    zi = nc.dram_tensor("zi_sh", [blk_i, Hd], F32, kind="ExternalOutput").ap()

    with tile.TileContext(nc) as tc:
        with (
            tc.tile_pool(name="const", bufs=1) as cp,
            tc.tile_pool(name="dram", bufs=1, space="DRAM") as dp,
            tc.tile_pool(name="sba", bufs=4) as sba,
            tc.tile_pool(name="sbb", bufs=3) as sbb,
            tc.tile_pool(name="sbc", bufs=3) as sbc,
            tc.tile_pool(name="sbx", bufs=2) as sbx,
            tc.tile_pool(name="pstr", bufs=3, space="PSUM") as ps_tr,
            tc.tile_pool(name="psmm", bufs=3, space="PSUM") as ps_mm,
            tc.tile_pool(name="psdn", bufs=1, space="PSUM") as ps_dn,
            tc.tile_pool(name="psag", bufs=1, space="PSUM") as ps_ag,
        ):
            uid = [0]

            def nm(base):
                uid[0] += 1
                return f"{base}_{uid[0]}"

            # ---- constants ----
            ident = cp.tile([P, P], F32, tag="ident", name="ident")
            from concourse.masks import make_identity
            make_identity(nc, ident[:])
            iota_i = cp.tile([P, P], I32, tag="iotai", name="iota_i")
            nc.gpsimd.iota(iota_i[:], pattern=[[1, P]], base=0, channel_multiplier=0)
            iota_row = cp.tile([P, P], F32, tag="iotaf", name="iota_row")
            nc.vector.tensor_copy(iota_row[:], iota_i[:])
            iotac_i = cp.tile([P, 1], I32, tag="iotaci", name="iotac_i")
            nc.gpsimd.iota(iotac_i[:], pattern=[[1, 1]], base=0, channel_multiplier=1)
            iota_col = cp.tile([P, 1], F32, tag="iotacf", name="iota_col")
            nc.vector.tensor_copy(iota_col[:], iotac_i[:])
            ident_r = cp.tile([P, P], F32R, tag="identr", name="ident_r")
            nc.vector.tensor_tensor(out=ident_r[:], in0=iota_col[:].to_broadcast([P, P]),
                                    in1=iota_row[:], op=OP.is_equal)
            mask_col = cp.tile([P, 1], F32, tag="maskc", name="mask_col")
            nc.vector.tensor_scalar(out=mask_col[:], in0=iota_col[:],
                                    scalar1=float(CW), scalar2=None, op0=OP.is_lt)

            def load_const(name, shape, bcast=False, dt_=F32):
                t = cp.tile(shape, dt_, tag=f"c_{name}", name=f"c_{name}")
                src = wd_in[name]
                if bcast:
                    src = src.to_broadcast(shape)
                nc.sync.dma_start(out=t[:], in_=src)
                return t

            Wp_u = load_const("Wp_user", [cfg.Du, Hd], dt_=F32R)
            bpc_u = load_const("bpc_user", [Hd, 1])
            Wp_i = load_const("Wp_item", [cfg.Di, Hd], dt_=F32R)
            bpc_i = load_const("bpc_item", [Hd, 1])
            Wf_u = load_const("Wf_user", [Hd, Hd], dt_=F32R)
            bf_u = load_const("bf_user", [P, Hd], bcast=True)
            Wf_i = load_const("Wf_item", [Hd, Hd], dt_=F32R)
            bf_i = load_const("bf_item", [P, Hd], bcast=True)
            lw = {}
            for r in ("rates", "rated"):
                for l in range(2):
                    d = dict(
                        Wl=load_const(f"Wl_{r}{l}", [Hd, HC], dt_=F32R),
                        Wr=load_const(f"Wr_{r}{l}", [Hd, HC], dt_=F32R),
                        att=load_const(f"att_{r}{l}", [P, HC], bcast=True),
                        bias=load_const(f"bias_{r}{l}", [P, C], bcast=True),
                        bl=load_const(f"bl_{r}{l}", [P, HC], bcast=True),
                    )
                    # blbrWe: rows 0..126 = bl+br broadcast, row 127 = We
                    t = cp.tile([P, HC], F32, tag=f"c_bw_{r}{l}", name=f"c_bw_{r}{l}")
                    nc.sync.dma_start(out=t[:CW, :],
                                      in_=wd_in[f"blbr_{r}{l}"].to_broadcast([CW, HC]))
                    nc.sync.dma_start(out=t[CW:P, :], in_=wd_in[f"We_{r}{l}"])
                    d["blbrWe"] = t
                    lw[r, l] = d

            # resident per-direction edge metadata (column layouts)
            meta_sb = {}
            for r, s in (("rates", sr), ("rated", sd)):
                gi = cp.tile([P, s.T], I32, tag=f"gi_{r}", name=f"gi_{r}")
                nc.sync.dma_start(out=gi[:], in_=gmeta[r]["gidx"])
                dcc = cp.tile([P, s.T], F32, tag=f"dc_{r}", name=f"dc_{r}")
                nc.sync.dma_start(out=dcc[:], in_=gmeta[r]["dstc"])
                ec = cp.tile([P, s.T], F32, tag=f"ec_{r}", name=f"ec_{r}")
                nc.sync.dma_start(out=ec[:], in_=gmeta[r]["erc"])
                meta_sb[r] = (gi, dcc, ec)

            # ---- DRAM internal tables ----
            huT0 = dp.tile([Hd, Wu_tab], F32R, tag="huT0", name="huT0")
            hiT0 = dp.tile([Hd, Wi_tab], F32R, tag="hiT0", name="hiT0")
            hu1_sh = dp.tile([blk_u, Hd], F32, tag="hu1sh", name="hu1sh")
            hi1_sh = dp.tile([blk_i, Hd], F32, tag="hi1sh", name="hi1sh")
            hu2_sh = dp.tile([blk_u, Hd], F32, tag="hu2sh", name="hu2sh")
            hi2_sh = dp.tile([blk_i, Hd], F32, tag="hi2sh", name="hi2sh")
            hu1_full = dp.tile([cfg.Nu, Hd], F32, addr_space="Shared", tag="hu1f", name="hu1f")
            hi1_full = dp.tile([cfg.Ni, Hd], F32, addr_space="Shared", tag="hi1f", name="hi1f")

            groups = [list(range(cfg.M))]

            # ---- phase 0: local-shard projections into transposed tables ----
            def emit_proj_T(xT_ap, W_tab, Din, Wp, bpc, outT):
                for c0 in range(0, W_tab, 512):
                    w = min(512, W_tab - c0)
                    xs = sbx.tile([Din, 512], F32R, tag="pxs", name=nm("pxs"))
                    nc.sync.dma_start(out=xs[:, :w], in_=xT_ap[:, c0 : c0 + w])
                    yp = ps_mm.tile([P, 512], F32, tag="mm", name=nm("pyp"))
                    nc.tensor.matmul(yp[:Hd, :w], lhsT=Wp[:Din, :],
                                     rhs=xs[:Din, :w], start=True, stop=True)
                    ya = sbx.tile([Hd, 512], F32R, tag="pya", name=nm("pya"))
                    nc.scalar.activation(ya[:Hd, :w], yp[:Hd, :w], AF.Relu, bias=bpc[:Hd, :])
                    nc.sync.dma_start(out=outT[:, c0 : c0 + w], in_=ya[:Hd, :w])

            emit_proj_T(xT_user, Wu_tab, cfg.Du, Wp_u, bpc_u, huT0)
            emit_proj_T(xT_item, Wi_tab, cfg.Di, Wp_i, bpc_i, hiT0)

            def allgather(sh, full):
                nc.gpsimd.collective_compute(
                    "AllGather", OP.bypass, replica_groups=groups,
                    ins=[sh.opt()], outs=[full.opt()])

            # ---- one GATv2 direction for one layer ----
            def emit_direction(r, s: DirSched, layer, tbl, xrT_tab, xr_rm_sh, out_sh, lwd):
                """layer=0: sources from pre-gathered xgT (+on-the-fly projection),
                xr side from transposed table xrT_tab.
                layer=1: sources gathered from row-major full table tbl, xr side
                from row-major local shard xr_rm_sh."""
                gi, dcc, ec = meta_sb[r]
                xgT = gmeta[r]["xgT"]
                Wl, Wr, att, bias, blbrWe, bl_bc = (lwd["Wl"], lwd["Wr"], lwd["att"],
                                                    lwd["bias"], lwd["blbrWe"], lwd["bl"])
                if layer == 0:
                    Wp, bpc, Dx = ((Wp_u, bpc_u, cfg.Du) if r == "rates"
                                   else (Wp_i, bpc_i, cfg.Di))
                tau = 0
                hugT_strip = None  # layer-0: projected sources, 4 tiles per strip
                for j, (tcnt, wd) in enumerate(zip(s.tiles, s.widths)):
                    # ---- chunk prologue: xr_chunk [128d, HC] ----
                    if layer == 0:
                        hiT = sba.tile([Hd, P], F32R, tag="hiT", name=nm("hiT"))
                        nc.sync.dma_start(out=hiT[:],
                                          in_=xrT_tab[:, j * CW : j * CW + P])
                    else:
                        hi_c = sbx.tile([P, Hd], F32, tag="hic", name=nm("hic"))
                        nc.gpsimd.memset(hi_c[:], 0.0)
                        nc.sync.dma_start(out=hi_c[:wd, :],
                                          in_=xr_rm_sh[j * CW : j * CW + wd, :])
                        tp = ps_tr.tile([P, P], F32, tag="tr", name=nm("tp"))
                        nc.tensor.transpose(tp[:], hi_c[:], ident[:])
                        hiT = sba.tile([Hd, P], F32R, tag="hiT", name=nm("hiT"))
                        nc.scalar.activation(hiT[:], tp[:], AF.Copy)
                    xr_ps = ps_tr.tile([P, HC], F32, tag="tr", name=nm("xrp"))
                    nc.tensor.matmul(xr_ps[:], lhsT=hiT[:], rhs=Wr[:],
                                     start=True, stop=True)
                    xr_sb = sbx.tile([P, HC], F32R, tag="xr", name=nm("xrs"))
                    nc.vector.scalar_tensor_tensor(
                        out=xr_sb[:], in0=xr_ps[:], scalar=mask_col[:],
                        in1=blbrWe[:], op0=OP.mult, op1=OP.add)

                    den_ps = ps_dn.tile([P, H], F32, tag="den", name=nm("den"))
                    agg_ps = ps_ag.tile([P, HC], F32, tag="agg", name=nm("agg"))

                    for t in range(tcnt):
                        # ---- source features, transposed: hu_gT [Hd, 128e] ----
                        if layer == 0:
                            if hugT_strip is None or tau % 4 == 0:
                                sw = min(4 * P, (s.T - (tau // 4) * 4) * P)
                                c0 = (tau // 4) * 4 * P
                                xg = sba.tile([Dx, 4 * P], F32R, tag="xg", name=nm("xg"))
                                nc.sync.dma_start(out=xg[:, :sw], in_=xgT[:, c0 : c0 + sw])
                                hp = ps_tr.tile([P, 4 * P], F32, tag="tr", name=nm("hp"))
                                nc.tensor.matmul(hp[:Hd, :sw], lhsT=Wp[:Dx, :],
                                                 rhs=xg[:Dx, :sw], start=True, stop=True)
                                hugT_strip = sba.tile([Hd, 4 * P], F32R, tag="hugT",
                                                      name=nm("hugT"))
                                nc.scalar.activation(hugT_strip[:, :sw], hp[:Hd, :sw],
                                                     AF.Relu, bias=bpc[:Hd, :])
                            o = (tau % 4) * P
                            hu_gT = hugT_strip[:, o : o + P]
                        else:
                            hu_g = sba.tile([P, Hd], F32, tag="hug", name=nm("hug"))
                            nc.gpsimd.indirect_dma_start(
                                out=hu_g[:], out_offset=None, in_=tbl[:, :],
                                in_offset=bass.IndirectOffsetOnAxis(
                                    ap=gi[:, tau : tau + 1], axis=0))
                            tp2 = ps_tr.tile([P, P], F32, tag="tr", name=nm("tp2"))
                            nc.tensor.transpose(tp2[:], hu_g[:], ident[:])
                            hu_gT_t = sba.tile([Hd, P], F32R, tag="hugT2", name=nm("hugT2"))
                            nc.scalar.activation(hu_gT_t[:], tp2[:], AF.Copy)
                            hu_gT = hu_gT_t[:, :]

                        # ---- one-hot matrices (slot 127 carries er) ----
                        msel = sba.tile([P, P], F32R, tag="msel", name=nm("msel"))
                        nc.vector.tensor_tensor(
                            out=msel[:], in0=dcc[:, tau : tau + 1].to_broadcast([P, P]),
                            in1=iota_row[:], op=OP.is_equal)
                        nc.vector.tensor_copy(msel[:, CW : CW + 1], ec[:, tau : tau + 1])
                        tp3 = ps_tr.tile([P, P], F32R, tag="tr", name=nm("tp3"))
                        nc.tensor.transpose(tp3[:], msel[:], ident_r[:])
                        mt = sba.tile([P, P], F32R, tag="mt", name=nm("mt"))
                        nc.scalar.activation(mt[:], tp3[:], AF.Copy)

                        # ---- matmuls ----
                        B0 = ps_mm.tile([P, HC], F32, tag="mm", name=nm("B0"))
                        nc.tensor.matmul(B0[:], lhsT=mt[:], rhs=xr_sb[:],
                                         start=True, stop=False)
                        nc.tensor.matmul(B0[:], lhsT=hu_gT, rhs=Wl[:],
                                         start=False, stop=True)
                        B1 = ps_mm.tile([P, HC], F32, tag="mm", name=nm("B1"))
                        nc.tensor.matmul(B1[:], lhsT=hu_gT, rhs=Wl[:],
                                         start=True, stop=True)

                        # ---- attention ----
                        g = sbb.tile([P, HC], F32, tag="g", name=nm("g"))
                        nc.scalar.activation(g[:], B0[:], AF.Prelu, alpha=NEG_SLOPE)
                        ga = sbb.tile([P, HC], F32, tag="ga", name=nm("ga"))
                        logits = sbc.tile([P, H], F32, tag="lg", name=nm("lg"))
                        for h in range(H):
                            hs = slice(h * C, (h + 1) * C)
                            nc.vector.scalar_tensor_tensor(
                                out=ga[:, hs], in0=g[:, hs], scalar=1.0,
                                in1=att[:, hs], op0=OP.mult, op1=OP.mult,
                                accum_out=logits[:, h : h + 1])
                        p = sbc.tile([P, H], F32, tag="p", name=nm("p"))
                        nc.scalar.activation(p[:], logits[:], AF.Exp)
                        w_all = sbb.tile([P, HC], F32R, tag="w", name=nm("w"))
                        for h in range(H):
                            hs = slice(h * C, (h + 1) * C)
                            nc.scalar.activation(w_all[:, hs], B1[:, hs], AF.Copy,
                                                 scale=p[:, h : h + 1])
                        p_r = sbc.tile([P, H], F32R, tag="pr", name=nm("pr"))
                        nc.vector.tensor_copy(p_r[:], p[:])
                        nc.tensor.matmul(den_ps[:], lhsT=msel[:], rhs=p_r[:],
                                         start=(t == 0), stop=(t == tcnt - 1))
                        nc.tensor.matmul(agg_ps[:], lhsT=msel[:], rhs=w_all[:],
                                         start=(t == 0), stop=(t == tcnt - 1))
                        tau += 1

                    # ---- chunk epilogue ----
                    den_sb = sbc.tile([P, H], F32, tag="dn", name=nm("dn"))
                    nc.vector.tensor_scalar_max(den_sb[:], den_ps[:], 1e-16)
                    rr = sbc.tile([P, H], F32, tag="rr", name=nm("rr"))
                    nc.vector.reciprocal(rr[:], den_sb[:])
                    rh = sbc.tile([P, H], F32, tag="rh", name=nm("rh"))
                    nc.vector.tensor_scalar_mul(rh[:], rr[:], 1.0 / H)
                    q = sbc.tile([P, H], F32, tag="q", name=nm("q"))
                    nc.vector.tensor_tensor(out=q[:], in0=den_ps[:], in1=rh[:], op=OP.mult)
                    acc2 = sbx.tile([P, HC], F32, tag="acc2", name=nm("acc2"))
                    prev = bias[:, 0:C]
                    for h in range(H):
                        hs = slice(h * C, (h + 1) * C)
                        nc.vector.scalar_tensor_tensor(
                            out=acc2[:, hs], in0=bl_bc[:, hs], scalar=q[:, h : h + 1],
                            in1=prev, op0=OP.mult, op1=OP.add)
                        prev = acc2[:, hs]
                    acc = sbx.tile([P, HC], F32, tag="acc", name=nm("acc"))
                    for h in range(H):
                        hs = slice(h * C, (h + 1) * C)
                        nc.vector.scalar_tensor_tensor(
                            out=acc[:, hs], in0=agg_ps[:, hs], scalar=rh[:, h : h + 1],
                            in1=prev, op0=OP.mult, op1=OP.add)
                        prev = acc[:, hs]
                    out_sb = sbx.tile([P, C], F32, tag="ou", name=nm("ou"))
                    nc.scalar.activation(out_sb[:], prev, AF.Relu)
                    nc.sync.dma_start(out=out_sh[j * CW : j * CW + wd, :],
                                      in_=out_sb[:wd, :])

            # layer 1 (sources pre-gathered; xr from transposed init tables)
            emit_direction("rates", sr, 0, None, hiT0, None, hi1_sh, lw["rates", 0])
            allgather(hi1_sh, hi1_full)
            emit_direction("rated", sd, 0, None, huT0, None, hu1_sh, lw["rated", 0])
            allgather(hu1_sh, hu1_full)
            # layer 2 (rated first: its gather table hi1_full is ready earlier)
            emit_direction("rated", sd, 1, hi1_full[:, :], None, hu1_sh, hu2_sh, lw["rated", 1])
            emit_direction("rates", sr, 1, hu1_full[:, :], None, hi1_sh, hi2_sh, lw["rates", 1])

            # ---- final projection + l2norm ----
            def emit_final(h_sh, n_rows, Wf, bf, z_out):
                n_ch = -(-n_rows // P)
                for j in range(n_ch):
                    r0 = j * P
                    wd = min(P, n_rows - r0)
                    h_c = sbx.tile([P, Hd], F32, tag="fhc", name=nm("fhc"))
                    if wd < P:
                        nc.gpsimd.memset(h_c[:], 0.0)
                    nc.sync.dma_start(out=h_c[:wd, :], in_=h_sh[r0 : r0 + wd, :])
                    tp = ps_tr.tile([P, P], F32, tag="tr", name=nm("ftp"))
                    nc.tensor.transpose(tp[:], h_c[:], ident[:])
                    hT = sba.tile([Hd, P], F32R, tag="fhT", name=nm("fhT"))
                    nc.scalar.activation(hT[:], tp[:], AF.Copy)
                    yp = ps_mm.tile([P, HC], F32, tag="mm", name=nm("fyp"))
                    nc.tensor.matmul(yp[:, :Hd], lhsT=hT[:], rhs=Wf[:],
                                     start=True, stop=True)
                    yb = sbx.tile([P, Hd], F32, tag="fyb", name=nm("fyb"))
                    nc.vector.tensor_add(yb[:], yp[:, :Hd], bf[:])
                    sq = sbx.tile([P, Hd], F32, tag="fsq", name=nm("fsq"))
                    ss = sbc.tile([P, 1], F32, tag="fss", name=nm("fss"))
                    nc.scalar.activation(sq[:], yb[:], AF.Square, accum_out=ss[:])
                    nrm = sbc.tile([P, 1], F32, tag="fnr", name=nm("fnr"))
                    nc.scalar.activation(nrm[:], ss[:], AF.Sqrt)
                    nmx = sbc.tile([P, 1], F32, tag="fnm", name=nm("fnm"))
                    nc.vector.tensor_scalar_max(nmx[:], nrm[:], 1e-12)
                    inv = sbc.tile([P, 1], F32, tag="fin", name=nm("fin"))
                    nc.vector.reciprocal(inv[:], nmx[:])
                    z_sb = sbx.tile([P, Hd], F32, tag="fz", name=nm("fz"))
                    nc.scalar.activation(z_sb[:], yb[:], AF.Copy, scale=inv[:])
                    nc.sync.dma_start(out=z_out[r0 : r0 + wd, :], in_=z_sb[:wd, :])

            emit_final(hu2_sh, blk_u, Wf_u, bf_u, zu)
            emit_final(hi2_sh, blk_i, Wf_i, bf_i, zi)

    nc.compile()
    return nc


def run_model(inputs, cfg: Cfg, trace=False, trace_kwargs=None):
    from concourse import bass_utils

    in_maps, sr, sd = prep_inputs(inputs, cfg)
    nc = build_program(cfg, sr, sd)
    res = bass_utils.run_bass_kernel_spmd(
        nc, in_maps, core_ids=list(range(cfg.M)), trace=trace,
        **(trace_kwargs or {}))
    outs = res.results
    zu = np.concatenate([outs[c]["zu_sh"] for c in range(cfg.M)], axis=0)
    zi = np.concatenate([outs[c]["zi_sh"] for c in range(cfg.M)], axis=0)
    return (zu, zi), res


def kernel(**inputs):
    cfg = Cfg()
    (zu, zi), _ = run_model(inputs, cfg, trace=False)
    return zu, zi
